# revision 48
# baseline (speedup 1.0000x reference)
"""MoE (8 experts, top-2, d=1024, N=8192) on 8 trn2 NeuronCores.

Strategy (capacity-balanced expert-parallel, mixed precision):
 - Host computes routing (top-2 expert ids + gates per token, fp64 logits for
   stable ordering). Token-expert pairs are split by gate magnitude: pairs
   with gate < THETA are computed in fp8-e4m3 with DoubleRow matmuls (2x PE
   throughput, K=256 per instruction); the rest in bf16. Each precision
   class is chopped into 128-token tiles per expert and distributed so every
   core gets exactly Tb bf16 tiles and T8 fp8 tiles. Per precision, each
   core holds TWO expert weight slots (primary/secondary); tiles 0..S-1 use
   slot 0 and the rest slot 1 (S is compile-time, experts are per-core DATA).
   A covering solver picks (T, S) and the assignment per precision.
 - Device (per core, SPMD): router logits for all tiles from bf16 x
   (replicated router), gate g = sigmoid((l_own+br_own) - max_{e!=own}) via
   two masked reduce_max; expert matmul per tile with PSUM K-accumulation
   (bf16: 16 N=512 matmuls; fp8: 8 DoubleRow N=512 matmuls, K=256 each),
   unscaled y emitted bf16 (fp8 tiles carry a x32 weight scale).
 - Host combines (weighted all-to-all): out[idx] += g*(y*inv_scale + b[e]).
"""

import os
from contextlib import ExitStack

import ml_dtypes
import numpy as np

import concourse.bass as bass
import concourse.bacc as bacc
import concourse.mybir as mybir
import concourse.tile as tile
from concourse.bass import ts
from concourse.bass_utils import run_bass_kernel_spmd

N_EXPERTS = 8
TOP_K = 2
D = 1024
N_CORES = 8
P = 128  # partitions
KT = D // P  # number of K tiles (8)
NH = 512  # psum free-dim tile (one bank of fp32)
EW = N_EXPERTS  # logit row width
N_TOKENS = 8192
RT = N_TOKENS // (N_CORES * P)  # router tiles per core (token-contiguous)
G = int(os.environ.get("MOE_G", "3"))  # token tiles per group
W8SCALE = 32.0  # fp8 weight pre-scale (W entries ~ N(0, 1/32))

MM_DTYPE = "bf16"
# gate thresholds: pairs with gate < THETA_LO run fp8 DoubleRow, pairs with
# gate >= THETA_HI run bf16; pairs in between are "flex" and are assigned to
# whichever class minimizes per-expert tile cost (bf16 tile = 2x fp8 tile)
THETA_LO = float(os.environ.get("MOE_THETA_LO", "0.40"))
THETA_HI = float(os.environ.get("MOE_THETA_HI", "0.50"))
# error budget: keep total rel err <= ERR_TARGET using the calibrated model
# err^2 = ERR_BF16^2 + ERR_C * sum(g^2 over fp8 pairs)
ERR_TARGET = float(os.environ.get("MOE_ERR_TARGET", "0.0182"))
ERR_BF16 = 3.2e-3
ERR_C = 3.15e-7
# ablation for HW bottleneck decomposition (bench-only, breaks correctness):
#   noload / nostore / pe (neither) / norouter / penr / dmaonly (no compute)
ABLATE = os.environ.get("MOE_ABLATE", "")
BUFS = int(os.environ.get("MOE_BUFS", "3"))  # x/y pool double-buffering depth
RCHUNK = int(os.environ.get("MOE_RCHUNK", "24"))  # router pairs emitted per group
YSTORE = os.environ.get("MOE_YSTORE", "half")  # y store granularity: group|tile|half
YQ = os.environ.get("MOE_YQ", "alt")  # y store queue: act|sync|pool|alt (sync/act per tile)
YPBUFS = int(os.environ.get("MOE_YPBUFS", "6"))  # expert psum pool depth
LPBUFS = int(os.environ.get("MOE_LPBUFS", "2"))  # router psum pool depth
# "end": queue all y stores at body end so their SBUF reads overlap the next
# iteration's router window instead of the port-saturated expert streams
YDELAY = os.environ.get("MOE_YDELAY", "0")
DRAINQ = os.environ.get("MOE_DRAINQ", "mix")  # psum drain engines: mix|dve
YXQ = os.environ.get("MOE_YXQ", "0")  # "1": cross-pair store queue vs drain engine
GORDER = os.environ.get("MOE_GORDER", "seq")  # group order: seq (b..b,8..8) | mix (round-robin)

LAST_RESULTS = None  # stash of BassKernelResults for test harness inspection

_BUILD_CACHE = {}


def _build(Tb: int, Sb: int, T8: int, S8: int, repeat: int = 1):
    """Build the SPMD Bass module: Tb bf16 + T8 fp8 token tiles per core."""
    key = (Tb, Sb, T8, S8, repeat, G, ABLATE, BUFS, RCHUNK, YSTORE, YQ, YPBUFS, LPBUFS, YDELAY, DRAINQ, YXQ, GORDER)
    if key in _BUILD_CACHE:
        return _BUILD_CACHE[key]

    f32 = mybir.dt.float32
    bf16 = mybir.dt.bfloat16
    fp8 = mybir.dt.float8e4
    T_all = Tb + T8

    nc = bacc.Bacc(None, target_bir_lowering=False)
    # bf16 tokens for the bf16 expert tiles:
    # xg_t[p, ((t*KT + j)*P + c)] = x[tile t slot c, j*128 + p]
    xg_t = nc.declare_dram_parameter("xg_t", [P, Tb * KT * P], bf16, isOutput=False)
    ws = nc.declare_dram_parameter("ws", [2 * D, D], bf16, isOutput=False)
    wr = nc.declare_dram_parameter("wr", [D, EW], bf16, isOutput=False)
    if T8 > 0:
        # fp8 tokens for fp8 tiles only (expert stationary)
        xg8 = nc.declare_dram_parameter("xg8", [P, T8 * KT * P], fp8, isOutput=False)
        ws8 = nc.declare_dram_parameter("ws8", [2 * D, D], fp8, isOutput=False)
    # router tokens: core-contiguous (token c*1024 + t*128 + p in slot
    # [p, (t j c)]); per-token masks: mro masks the token's top-1 column,
    # mrw masks all but the top-1 column (rows differ per partition)
    xr = nc.declare_dram_parameter("xr", [P, RT * KT * P], bf16, isOutput=False)
    mro = nc.declare_dram_parameter("mro", [P, RT * EW], f32, isOutput=False)
    mrw = nc.declare_dram_parameter("mrw", [P, RT * EW], f32, isOutput=False)
    # outputs (y tiled [T_all, 128 tok, 1024 feat]; primary gates per token)
    y_dt = f32 if YSTORE == "psum" else bf16
    y = nc.declare_dram_parameter("y", [P, T_all * D], y_dt, isOutput=True)
    gout = nc.declare_dram_parameter("gout", [P, RT], f32, isOutput=True)

    with tile.TileContext(nc) as tc, ExitStack() as ctx:
        consts = ctx.enter_context(tc.tile_pool(name="consts", bufs=1))
        gpool = ctx.enter_context(tc.tile_pool(name="gates", bufs=2))
        gspool = ctx.enter_context(tc.tile_pool(name="gsb", bufs=2))
        ypool = ctx.enter_context(tc.tile_pool(name="y", bufs=BUFS))
        lpsum = ctx.enter_context(
            tc.tile_pool(name="lpsum", bufs=LPBUFS, space=bass.MemorySpace.PSUM)
        )
        ypsum = ctx.enter_context(
            tc.tile_pool(name="ypsum", bufs=YPBUFS, space=bass.MemorySpace.PSUM)
        )

        # ---- constants / weights resident in SBUF ----
        w_sb = consts.tile([P, 2, KT, D], bf16)
        nc.sync.dma_start(w_sb[:], ws.rearrange("(s kt p) n -> p s kt n", p=P, s=2))
        if T8 > 0:
            w8_sb = consts.tile([P, 2, KT, D], fp8)
            nc.sync.dma_start(
                w8_sb[:], ws8.rearrange("(s kt p) n -> p s kt n", p=P, s=2)
            )

        wr_sb = consts.tile([P, KT, EW], bf16)
        nc.sync.dma_start(wr_sb[:], wr.rearrange("(kt p) n -> p kt n", p=P))

        mro_sb = consts.tile([P, RT * EW], f32)
        nc.sync.dma_start(mro_sb[:], mro[:, :])
        mrw_sb = consts.tile([P, RT * EW], f32)
        nc.sync.dma_start(mrw_sb[:], mrw[:, :])

        # token data resident in SBUF (staged once, like the weights)
        xg_sb = consts.tile([P, Tb * KT * P], bf16)
        nc.sync.dma_start(xg_sb[:], xg_t[:, :])
        xr_sb = consts.tile([P, RT * KT * P], bf16)
        nc.sync.dma_start(xr_sb[:], xr[:, :])
        if T8 > 0:
            xg8_sb = consts.tile([P, T8 * KT * P], fp8)
            nc.sync.dma_start(xg8_sb[:], xg8[:, :])

        rep_cm = None
        if repeat > 1:
            rep_cm = tc.For_i(0, repeat, 1)
            rep_cm.__enter__()

        # groups: (kind, g0, gt) with g0 a GLOBAL tile index; fp8 groups
        # follow the bf16 groups and never straddle the boundary.
        groups_b = [("b", g0, min(G, Tb - g0)) for g0 in range(0, Tb, G)]
        groups_8 = [("8", Tb + g0, min(G, T8 - g0)) for g0 in range(0, T8, G)]
        if GORDER == "mix":
            groups = []
            for i in range(max(len(groups_b), len(groups_8))):
                if i < len(groups_b):
                    groups.append(groups_b[i])
                if i < len(groups_8):
                    groups.append(groups_8[i])
        else:
            groups = groups_b + groups_8

        do_load = ABLATE not in ("noload", "pe", "penr")
        do_store = ABLATE not in ("nostore", "pe", "penr")
        do_pe = ABLATE != "dmaonly"
        do_router = do_pe and ABLATE not in ("norouter", "penr")
        kt_eff = KT // 2 if ABLATE == "halfk" else KT

        # router emission state machine: one tiny (LDW + 8-row matmul) pair at
        # a time, interleaved between 512-row expert matmuls so the router
        # stationary loads hide under the long streams
        r_state = {"pos": 0, "lp": None, "gsb": None}
        n_router_ops = RT * KT

        def router_chain(rtau, Lp, gsb):
            """Gate chain for one router tile (two masked reduce_max)."""
            Lm1 = gpool.tile([P, EW], f32, tag="lm1")
            nc.vector.tensor_add(Lm1[:], Lp[:], mro_sb[:, rtau * EW : (rtau + 1) * EW])
            Lm2 = gpool.tile([P, EW], f32, tag="lm2")
            nc.vector.tensor_add(Lm2[:], Lp[:], mrw_sb[:, rtau * EW : (rtau + 1) * EW])
            mo = gpool.tile([P, 1], f32, tag="mo")
            nc.vector.reduce_max(
                mo[:], Lm1[:].rearrange("p (g e) -> p g e", e=EW),
                axis=mybir.AxisListType.X,
            )
            so = gpool.tile([P, 1], f32, tag="so")
            nc.vector.reduce_max(
                so[:], Lm2[:].rearrange("p (g e) -> p g e", e=EW),
                axis=mybir.AxisListType.X,
            )
            nc.vector.tensor_sub(so[:], so[:], mo[:])
            nc.scalar.activation(
                gsb[:, rtau : rtau + 1], so[:], mybir.ActivationFunctionType.Sigmoid
            )

        def emit_router(n):
            """Emit up to n router matmul pairs (chains piggyback for free)."""
            if not do_router:
                return
            emitted = 0
            while r_state["pos"] < n_router_ops and emitted < n:
                pos = r_state["pos"]
                rtau, j = divmod(pos, KT)
                if j == 0:
                    r_state["lp"] = lpsum.tile([P, EW], f32, tag="lp", name="lp")
                nc.tensor.matmul(
                    r_state["lp"][:],
                    xr_sb[:, (rtau * KT + j) * P : (rtau * KT + j + 1) * P],
                    wr_sb[:, j, :],
                    start=(j == 0),
                    stop=(j == KT - 1),
                )
                if j == KT - 1:
                    router_chain(rtau, r_state["lp"], r_state["gsb"])
                r_state["pos"] = pos + 1
                emitted += 1

        def experts(kind, g0, gt, gsb):
            ysb = None
            if YSTORE != "psum":
                ysb = ypool.tile([P, gt * D], bf16, tag="ysb")
            if T8 > 0:
                x8v = xg8_sb[:].rearrange("p (t k c) -> p t k c", k=KT, c=P)
            for tau in range(gt if do_pe else 0):
                t = g0 + tau
                yph = [
                    ypsum.tile([P, NH], f32, tag="yph", name="yph")
                    for _ in range(2)
                ]
                if kind == "b":
                    slot = 0 if t < Sb else 1
                    for nh in range(2):
                        for j in range(kt_eff):
                            nc.tensor.matmul(
                                yph[nh][:],
                                xg_sb[:, (t * KT + j) * P : (t * KT + j + 1) * P],
                                w_sb[:, slot, j, ts(nh, NH)],
                                start=(j == 0),
                                stop=(j == kt_eff - 1),
                            )
                else:
                    t8 = t - Tb
                    slot = 0 if t8 < S8 else 1
                    kp_eff = kt_eff // 2
                    for nh in range(2):
                        for m in range(kp_eff):
                            nc.tensor.matmul(
                                yph[nh][:],
                                x8v[:, t8, 2 * m : 2 * m + 2, :],
                                w8_sb[:, slot, 2 * m : 2 * m + 2, ts(nh, NH)],
                                start=(m == 0),
                                stop=(m == kp_eff - 1),
                                perf_mode=mybir.MatmulPerfMode.DoubleRow,
                            )
                if YSTORE == "psum":
                    if do_store:
                        eng = nc.sync if t % 2 == 0 else nc.scalar
                        for nh in range(2):
                            eng.dma_start(
                                y[:, t * D + nh * NH : t * D + (nh + 1) * NH],
                                yph[nh][:],
                            )
                    else:
                        # keep psum consumed so the pool can recycle
                        junk = gpool.tile([P, 1], f32, tag="junk", name="junk")
                        nc.vector.reduce_max(junk[:], yph[0][:], axis=mybir.AxisListType.X)
                        nc.vector.reduce_max(junk[:], yph[1][:], axis=mybir.AxisListType.X)
                    continue
                for nh in range(2):
                    dsth = ysb[:, tau * D + nh * NH : tau * D + (nh + 1) * NH]
                    if nh == 0 or DRAINQ == "dve":
                        nc.vector.tensor_copy(dsth, yph[nh][:])
                    else:
                        nc.scalar.copy(dsth, yph[nh][:])
            def emit_store():
                eng = {"act": nc.scalar, "sync": nc.sync, "pool": nc.gpsimd}.get(YQ, nc.scalar)
                if YSTORE == "half":
                    for tau in range(gt):
                        for nh in range(2):
                            k = (g0 + tau) * 2 + nh + (1 if YXQ == "1" else 0)
                            e2 = nc.sync if k % 2 == 0 else nc.scalar
                            e2.dma_start(
                                y[:, (g0 + tau) * D + nh * NH : (g0 + tau) * D + (nh + 1) * NH],
                                ysb[:, tau * D + nh * NH : tau * D + (nh + 1) * NH],
                            )
                elif YSTORE == "tile":
                    for tau in range(gt):
                        qs = {"alt": [nc.sync, nc.scalar],
                              "alt3": [nc.sync, nc.scalar, nc.gpsimd],
                              "alt4": [nc.sync, nc.scalar, nc.gpsimd, nc.vector]}.get(YQ)
                        e2 = qs[(g0 + tau) % len(qs)] if qs else eng
                        e2.dma_start(
                            y[:, (g0 + tau) * D : (g0 + tau + 1) * D],
                            ysb[:, tau * D : (tau + 1) * D],
                        )
                else:
                    eng.dma_start(y[:, g0 * D : (g0 + gt) * D], ysb[:])

            if do_store and YSTORE != "psum":
                if YDELAY == "end":
                    pending_stores.append(emit_store)
                else:
                    emit_store()

        pending_stores = []
        gsb_cur = gspool.tile([P, RT], f32, tag="gsb", name="gsb")
        r_state["gsb"] = gsb_cur
        if not do_router:
            nc.gpsimd.memset(gsb_cur[:], 1.0)
        for i, (kind, g0, gt) in enumerate(groups):
            emit_router(RCHUNK)  # a few router tiles' worth per group
            experts(kind, g0, gt, gsb_cur)
            if i + 1 == len(groups):
                emit_router(n_router_ops)  # flush any remaining router work
                for st in pending_stores:
                    st()
                pending_stores.clear()
                if do_pe:
                    nc.sync.dma_start(gout[:, :], gsb_cur[:])

        if rep_cm is not None:
            rep_cm.__exit__(None, None, None)

    nc.compile()
    _BUILD_CACHE[key] = nc
    return nc


def _route(x, Wr, br):
    """Host routing in fp64: per-token top-2 expert ids + gates."""
    n_tokens = x.shape[0]
    logits = x.astype(np.float64) @ Wr.astype(np.float64) + br.astype(np.float64)
    i1 = np.argmax(logits, axis=1)
    l1 = logits[np.arange(n_tokens), i1]
    l2m = logits.copy()
    l2m[np.arange(n_tokens), i1] = -np.inf
    i2 = np.argmax(l2m, axis=1)
    l2 = logits[np.arange(n_tokens), i2]
    g1 = 1.0 / (1.0 + np.exp(l2 - l1))
    return i1, i2, g1, 1.0 - g1


def _solve_assignment(n_tiles_per_e):
    """Pick (T, S) and per-core (primary_e, secondary_e) so that 8 units of
    size S plus 8 units of size T-S cover the per-expert tile demands.
    Exact DP over (S-units used, (T-S)-units used).

    Returns (T, S, pri_experts[8], sec_experts[8]) or None."""
    total = int(sum(n_tiles_per_e))
    if total == 0:
        return 0, 0, [0] * N_CORES, [0] * N_CORES
    t_min = max(1, (total + N_CORES - 1) // N_CORES)
    for T in range(t_min, t_min + 4):
        for S in range((T + 1) // 2, T + 1):
            sz2 = T - S
            # per-expert candidate (p, q) unit counts, Pareto-minimal
            opts = []
            for ne in n_tiles_per_e:
                cand = []
                for p in range(N_CORES + 1):
                    need = ne - S * p
                    q = 0 if need <= 0 else (
                        (need + sz2 - 1) // sz2 if sz2 > 0 else None
                    )
                    if q is None or q > N_CORES:
                        continue
                    if any(p2 <= p and q2 <= q for p2, q2 in cand):
                        continue
                    cand = [(p2, q2) for p2, q2 in cand if not (p <= p2 and q <= q2)]
                    cand.append((p, q))
                if not cand:
                    cand = None
                opts.append(cand)
            if any(o is None for o in opts):
                continue
            # DP: state (pu, qu) -> choice list
            states = {(0, 0): []}
            for cand in opts:
                nxt = {}
                for (pu, qu), hist in states.items():
                    for p, q in cand:
                        k = (pu + p, qu + q)
                        if k[0] <= N_CORES and k[1] <= N_CORES and k not in nxt:
                            nxt[k] = hist + [(p, q)]
                states = nxt
            if not states:
                continue
            (pu, qu), hist = min(states.items(), key=lambda kv: kv[0])
            pri, sec = [], []
            for e, (p, q) in enumerate(hist):
                pri += [e] * p
                sec += [e] * q
            pri += [0] * (N_CORES - len(pri))  # leftover units: pure padding
            sec += [0] * (N_CORES - len(sec))
            return T, S, pri, sec
    return None


def _assign(idx_per_e):
    """Tile one precision class and distribute across cores.

    Returns (T, S, core_tiles) with core_tiles[c] a list of (expert, idx)."""
    tiles_per_e = [
        [idx[i : i + P] for i in range(0, len(idx), P)] for idx in idx_per_e
    ]
    n_tiles = [len(tl) for tl in tiles_per_e]
    sol = _solve_assignment(n_tiles)
    if sol is None:
        T = max(max(n_tiles), 1)
        S = T
        pri = list(range(N_CORES))
        sec = list(range(N_CORES))
    else:
        T, S, pri, sec = sol
    queues = [list(tl) for tl in tiles_per_e]
    core_tiles = [[] for _ in range(N_CORES)]
    for c in range(N_CORES):
        for e, cnt in ((pri[c], S), (sec[c], T - S)):
            for _ in range(cnt):
                idx = queues[e].pop(0) if queues[e] else np.empty(0, np.int64)
                core_tiles[c].append((e, idx))
    assert all(not q for q in queues), "assignment failed to place all tiles"
    return T, S, core_tiles


def _plan(x, Wr, br, W, b):
    i1, i2, g1, g2 = _route(x, Wr, br)
    n = x.shape[0]
    # token-expert pairs: (expert, token, gate)
    pairs_e = np.concatenate([i1, i2])
    pairs_t = np.concatenate([np.arange(n), np.arange(n)])
    pairs_g = np.concatenate([g1, g2])
    # per-expert flex split under a global error budget:
    # choose k_e (lowest-gate k_e pairs -> fp8) minimizing PE cost
    # 2*ceil(nb/128) + ceil(n8/128) subject to sum of fp8 g^2 <= budget
    budget = max(0.0, (ERR_TARGET**2 - ERR_BF16**2) / ERR_C)
    toks, gsq, f8s, his, ks = [], [], [], [], []
    for e in range(N_EXPERTS):
        m = pairs_e == e
        t_e = pairs_t[m]
        g_e = pairs_g[m]
        order = np.argsort(g_e, kind="stable")
        toks.append(t_e[order])
        gs = np.concatenate([[0.0], np.cumsum(g_e[order] ** 2)])
        gsq.append(gs)
        f8s.append(int((g_e < THETA_LO).sum()))
        his.append(int((g_e < THETA_HI).sum()))
        ks.append(f8s[-1])

    def cover_cost(kvec):
        """Per-core PE cost of the full covering for a split: 2*Tb + T8
        (bf16 tile = 16 matmuls, fp8 tile = 8). Returns (cost, Tb, T8)."""
        nb = [-(-(len(toks[e]) - kvec[e]) // P) for e in range(N_EXPERTS)]
        n8 = [-(-kvec[e] // P) for e in range(N_EXPERTS)]
        sb_ = _solve_assignment(nb)
        s8_ = _solve_assignment(n8)
        if sb_ is None or s8_ is None:
            return None
        return 2 * sb_[0] + s8_[0], sb_[0], s8_[0]

    # candidate splits: sharp thresholds + the per-expert greedy trajectory
    # (objective 2*ceil(nb/128)+ceil(n8/128)); evaluate each with the real
    # covering solver and keep the cheapest within the error budget.
    g_sorted = [np.sort(pairs_g[pairs_e == e]) for e in range(N_EXPERTS)]
    cands = []
    for th in np.arange(THETA_LO, THETA_HI + 1e-9, 0.00625):
        cands.append(
            ([int(np.searchsorted(g_sorted[e], th)) for e in range(N_EXPERTS)], True)
        )
    # uniform-m candidates: keep the 128*m highest-gate pairs per expert in
    # bf16 (exact Tb=m cover with zero fragmentation), everything else fp8
    for m_u in range(7, 13):
        kvec = [max(0, len(toks[e]) - 128 * m_u) for e in range(N_EXPERTS)]
        cands.append((kvec, False))

    def pcost(e, k):
        n_e = len(toks[e])
        return 2 * -(-(n_e - k) // P) + -(-k // P)

    kg = list(ks)
    cands.append((list(kg), True))
    while True:
        best = None
        for e in range(N_EXPERTS):
            c0 = pcost(e, kg[e])
            for k in range(kg[e] + 1, his[e] + 1):
                if pcost(e, k) < c0:
                    ds = gsq[e][k] - gsq[e][kg[e]]
                    dc = c0 - pcost(e, k)
                    if best is None or ds / dc < best[0]:
                        best = (ds / dc, e, k)
                    break
        if best is None:
            break
        kg[best[1]] = best[2]
        cands.append((list(kg), True))

    best = None
    for kvec, clamp in cands:
        if clamp:
            kvec = [min(max(kvec[e], f8s[e]), his[e]) for e in range(N_EXPERTS)]
        spent = sum(gsq[e][kvec[e]] for e in range(N_EXPERTS))
        if spent > budget:
            continue
        cc = cover_cost(kvec)
        if cc is None:
            continue
        key = (cc[0], spent)
        if best is None or key < best[0]:
            best = (key, kvec)
    assert best is not None, "no feasible precision split"
    ks = best[1]
    idxb = [np.sort(toks[e][ks[e] :]) for e in range(N_EXPERTS)]
    idx8 = [np.sort(toks[e][: ks[e]]) for e in range(N_EXPERTS)]
    Tb, Sb, tiles_b = _assign(idxb)
    T8, S8, tiles_8 = _assign(idx8)
    plan = {
        "build_args": {"Tb": Tb, "Sb": Sb, "T8": T8, "S8": S8},
        "tiles_b": tiles_b,
        "tiles_8": tiles_8,
        "n_tokens": n,
        "b": b,
        "i1": i1,
    }
    return plan


def _make_in_maps(x, Wr, br, W, b, plan):
    Tb, Sb = plan["build_args"]["Tb"], plan["build_args"]["Sb"]
    T8, S8 = plan["build_args"]["T8"], plan["build_args"]["S8"]
    T_all = Tb + T8
    x8_full = x.astype(ml_dtypes.float8_e4m3)
    i1 = plan["i1"]
    in_maps = []
    for c in range(N_CORES):
        xg = np.zeros((Tb * P, D), dtype=np.float32)
        for t, (e, idx) in enumerate(plan["tiles_b"][c]):
            if len(idx):
                xg[t * P : t * P + len(idx)] = x[idx]
        # partition-major: xg_t[p, t, j, c] = xg[t*128 + c, j*128 + p]
        xg_t = np.ascontiguousarray(
            xg.reshape(Tb, P, KT, P).transpose(3, 0, 2, 1).reshape(P, Tb * KT * P)
        ).astype(ml_dtypes.bfloat16)
        # router tokens: core-contiguous slab, plus per-token top-1 masks
        toks = np.arange(c * RT * P, (c + 1) * RT * P)
        xr_t = np.ascontiguousarray(
            x[toks].reshape(RT, P, KT, P).transpose(3, 0, 2, 1).reshape(P, RT * KT * P)
        ).astype(ml_dtypes.bfloat16)
        own = i1[toks].reshape(RT, P)
        tt, pp = np.mgrid[0:RT, 0:P]
        mro = np.broadcast_to(br, (RT, P, EW)).astype(np.float32).copy()
        mro[tt, pp, own] = -1e30
        mrw = np.full((RT, P, EW), -1e30, np.float32)
        mrw[tt, pp, own] = br[own]
        tb = plan["tiles_b"][c]
        e_pri = tb[0][0] if Tb else 0
        e_sec = tb[-1][0] if Tb else 0
        ws = np.concatenate([W[e_pri], W[e_sec]], axis=0).astype(ml_dtypes.bfloat16)
        im = {
            "xg_t": xg_t,
            "xr": xr_t,
            "ws": np.ascontiguousarray(ws),
            "wr": np.ascontiguousarray(Wr).astype(ml_dtypes.bfloat16),
            "mro": np.ascontiguousarray(mro.transpose(1, 0, 2).reshape(P, RT * EW)),
            "mrw": np.ascontiguousarray(mrw.transpose(1, 0, 2).reshape(P, RT * EW)),
        }
        if T8 > 0:
            t8 = plan["tiles_8"][c]
            xg8 = np.zeros((T8 * P, D), dtype=ml_dtypes.float8_e4m3)
            for t, (e, idx) in enumerate(t8):
                if len(idx):
                    xg8[t * P : t * P + len(idx)] = x8_full[idx]
            im["xg8"] = np.ascontiguousarray(
                xg8.reshape(T8, P, KT, P).transpose(3, 0, 2, 1).reshape(P, T8 * KT * P)
            )
            e8_pri = t8[0][0]
            e8_sec = t8[-1][0]
            ws8 = np.concatenate([W[e8_pri], W[e8_sec]], axis=0) * W8SCALE
            im["ws8"] = np.ascontiguousarray(ws8).astype(ml_dtypes.float8_e4m3)
        in_maps.append(im)
    return in_maps


def _prep(inputs):
    x = np.asarray(inputs["x"], dtype=np.float32)
    Wr = np.asarray(inputs["Wr"], dtype=np.float32)
    br = np.asarray(inputs["br"], dtype=np.float32)
    W = np.asarray(inputs["W"], dtype=np.float32)
    b = np.asarray(inputs["b"], dtype=np.float32)
    plan = _plan(x, Wr, br, W, b)
    # sanity: slot boundary must match data layout (tile t uses slot 0 iff t<S)
    for c in range(N_CORES):
        for tiles, S in ((plan["tiles_b"][c], plan["build_args"]["Sb"]),
                         (plan["tiles_8"][c], plan["build_args"]["S8"])):
            for t, (e, _) in enumerate(tiles):
                want = tiles[0][0] if t < S else tiles[-1][0]
                assert e == want, (c, t, e, want)
    in_maps = _make_in_maps(x, Wr, br, W, b, plan)
    return in_maps, plan


def kernel(**inputs) -> np.ndarray:
    global LAST_RESULTS
    in_maps, plan = _prep(inputs)
    nc = _build(**plan["build_args"])
    res = run_bass_kernel_spmd(nc, in_maps, core_ids=list(range(N_CORES)))
    LAST_RESULTS = res

    Tb = plan["build_args"]["Tb"]
    T8 = plan["build_args"]["T8"]
    T_all = Tb + T8
    n_tokens, b, i1 = plan["n_tokens"], plan["b"], plan["i1"]
    # primary gate per token from the per-core router outputs
    g_all = np.concatenate(
        [res.results[c]["gout"].T.reshape(RT * P).astype(np.float32)
         for c in range(N_CORES)]
    )
    out = np.zeros((n_tokens, D), dtype=np.float32)
    for c in range(N_CORES):
        # y [P, T_all*D]: y[p, t*D + f] = tile t, token-slot p, feature f
        ye = (
            res.results[c]["y"]
            .reshape(P, T_all, D)
            .transpose(1, 0, 2)
            .astype(np.float32)
        )
        for tiles, scale in ((plan["tiles_b"][c], 1.0),
                             (plan["tiles_8"][c], 1.0 / W8SCALE)):
            off = 0 if scale == 1.0 else Tb
            for t, (e, idx) in enumerate(tiles):
                nn = len(idx)
                if nn:
                    g = np.where(i1[idx] == e, g_all[idx], 1.0 - g_all[idx])
                    out[idx] += g[:, None] * (
                        ye[off + t, :nn] * scale + b[e][None, :]
                    )
    return out


# revision 49
# speedup vs baseline: 1.0312x; 1.0312x over previous
"""MoE (8 experts, top-2, d=1024, N=8192) on 8 trn2 NeuronCores.

Strategy (capacity-balanced expert-parallel, mixed precision):
 - Host computes routing (top-2 expert ids + gates per token, fp64 logits for
   stable ordering). Token-expert pairs are split by gate magnitude: pairs
   with gate < THETA are computed in fp8-e4m3 with DoubleRow matmuls (2x PE
   throughput, K=256 per instruction); the rest in bf16. Each precision
   class is chopped into 128-token tiles per expert and distributed so every
   core gets exactly Tb bf16 tiles and T8 fp8 tiles. Per precision, each
   core holds TWO expert weight slots (primary/secondary); tiles 0..S-1 use
   slot 0 and the rest slot 1 (S is compile-time, experts are per-core DATA).
   A covering solver picks (T, S) and the assignment per precision.
 - Device (per core, SPMD): router logits for all tiles from bf16 x
   (replicated router), gate g = sigmoid((l_own+br_own) - max_{e!=own}) via
   two masked reduce_max; expert matmul per tile with PSUM K-accumulation
   (bf16: 16 N=512 matmuls; fp8: 8 DoubleRow N=512 matmuls, K=256 each),
   unscaled y emitted bf16 (fp8 tiles carry a x32 weight scale).
 - Host combines (weighted all-to-all): out[idx] += g*(y*inv_scale + b[e]).
"""

import os
from contextlib import ExitStack

import ml_dtypes
import numpy as np

import concourse.bass as bass
import concourse.bacc as bacc
import concourse.mybir as mybir
import concourse.tile as tile
from concourse.bass import ts
from concourse.bass_utils import run_bass_kernel_spmd

N_EXPERTS = 8
TOP_K = 2
D = 1024
N_CORES = 8
P = 128  # partitions
KT = D // P  # number of K tiles (8)
NH = 512  # psum free-dim tile (one bank of fp32)
EW = N_EXPERTS  # logit row width
N_TOKENS = 8192
RT = N_TOKENS // (N_CORES * P)  # router tiles per core (token-contiguous)
G = int(os.environ.get("MOE_G", "3"))  # token tiles per group
W8SCALE = 32.0  # fp8 weight pre-scale (W entries ~ N(0, 1/32))

MM_DTYPE = "bf16"
# gate thresholds: pairs with gate < THETA_LO run fp8 DoubleRow, pairs with
# gate >= THETA_HI run bf16; pairs in between are "flex" and are assigned to
# whichever class minimizes per-expert tile cost (bf16 tile = 2x fp8 tile)
THETA_LO = float(os.environ.get("MOE_THETA_LO", "0.40"))
THETA_HI = float(os.environ.get("MOE_THETA_HI", "0.50"))
# error budget: keep total rel err <= ERR_TARGET using the calibrated model
# err^2 = ERR_BF16^2 + ERR_C * sum(g^2 over fp8 pairs)
ERR_TARGET = float(os.environ.get("MOE_ERR_TARGET", "0.0182"))
ERR_BF16 = 3.2e-3
ERR_C = 3.15e-7
# ablation for HW bottleneck decomposition (bench-only, breaks correctness):
#   noload / nostore / pe (neither) / norouter / penr / dmaonly (no compute)
ABLATE = os.environ.get("MOE_ABLATE", "")
BUFS = int(os.environ.get("MOE_BUFS", "3"))  # x/y pool double-buffering depth
RCHUNK = int(os.environ.get("MOE_RCHUNK", "24"))  # router pairs emitted per group
YSTORE = os.environ.get("MOE_YSTORE", "half")  # y store granularity: group|tile|half
YQ = os.environ.get("MOE_YQ", "alt")  # y store queue: act|sync|pool|alt (sync/act per tile)
YPBUFS = int(os.environ.get("MOE_YPBUFS", "6"))  # expert psum pool depth
LPBUFS = int(os.environ.get("MOE_LPBUFS", "2"))  # router psum pool depth
# "end": queue all y stores at body end so their SBUF reads overlap the next
# iteration's router window instead of the port-saturated expert streams
YDELAY = os.environ.get("MOE_YDELAY", "0")
DRAINQ = os.environ.get("MOE_DRAINQ", "mix")  # psum drain engines: mix|dve
YXQ = os.environ.get("MOE_YXQ", "0")  # "1": cross-pair store queue vs drain engine
GORDER = os.environ.get("MOE_GORDER", "seq")  # group order: seq (b..b,8..8) | mix (round-robin)

LAST_RESULTS = None  # stash of BassKernelResults for test harness inspection

_BUILD_CACHE = {}


def _build(Tb: int, Sb: int, T8: int, S8: int, repeat: int = 1):
    """Build the SPMD Bass module: Tb bf16 + T8 fp8 token tiles per core."""
    key = (Tb, Sb, T8, S8, repeat, G, ABLATE, BUFS, RCHUNK, YSTORE, YQ, YPBUFS, LPBUFS, YDELAY, DRAINQ, YXQ, GORDER)
    if key in _BUILD_CACHE:
        return _BUILD_CACHE[key]

    f32 = mybir.dt.float32
    bf16 = mybir.dt.bfloat16
    fp8 = mybir.dt.float8e4
    T_all = Tb + T8

    nc = bacc.Bacc(None, target_bir_lowering=False)
    # bf16 tokens for the bf16 expert tiles:
    # xg_t[p, ((t*KT + j)*P + c)] = x[tile t slot c, j*128 + p]
    xg_t = nc.declare_dram_parameter("xg_t", [P, Tb * KT * P], bf16, isOutput=False)
    ws = nc.declare_dram_parameter("ws", [2 * D, D], bf16, isOutput=False)
    wr = nc.declare_dram_parameter("wr", [D, EW], bf16, isOutput=False)
    if T8 > 0:
        # fp8 tokens for fp8 tiles only (expert stationary)
        xg8 = nc.declare_dram_parameter("xg8", [P, T8 * KT * P], fp8, isOutput=False)
        ws8 = nc.declare_dram_parameter("ws8", [2 * D, D], fp8, isOutput=False)
    # router tokens: core-contiguous (token c*1024 + t*128 + p in slot
    # [p, (t j c)]); per-token masks: mro masks the token's top-1 column,
    # mrw masks all but the top-1 column (rows differ per partition)
    xr = nc.declare_dram_parameter("xr", [P, RT * KT * P], bf16, isOutput=False)
    mro = nc.declare_dram_parameter("mro", [P, RT * EW], f32, isOutput=False)
    mrw = nc.declare_dram_parameter("mrw", [P, RT * EW], f32, isOutput=False)
    # outputs (y tiled [T_all, 128 tok, 1024 feat]; primary gates per token)
    y_dt = f32 if YSTORE == "psum" else bf16
    y = nc.declare_dram_parameter("y", [P, T_all * D], y_dt, isOutput=True)
    gout = nc.declare_dram_parameter("gout", [P, RT], f32, isOutput=True)

    with tile.TileContext(nc) as tc, ExitStack() as ctx:
        consts = ctx.enter_context(tc.tile_pool(name="consts", bufs=1))
        gpool = ctx.enter_context(tc.tile_pool(name="gates", bufs=2))
        gspool = ctx.enter_context(tc.tile_pool(name="gsb", bufs=2))
        ypool = ctx.enter_context(tc.tile_pool(name="y", bufs=BUFS))
        lpsum = ctx.enter_context(
            tc.tile_pool(name="lpsum", bufs=LPBUFS, space=bass.MemorySpace.PSUM)
        )
        ypsum = ctx.enter_context(
            tc.tile_pool(name="ypsum", bufs=YPBUFS, space=bass.MemorySpace.PSUM)
        )

        # ---- constants / weights resident in SBUF ----
        # staging DMAs spread across three queues so startup overlaps
        w_sb = consts.tile([P, 2, KT, D], bf16)
        nc.sync.dma_start(w_sb[:], ws.rearrange("(s kt p) n -> p s kt n", p=P, s=2))
        if T8 > 0:
            w8_sb = consts.tile([P, 2, KT, D], fp8)
            nc.scalar.dma_start(
                w8_sb[:], ws8.rearrange("(s kt p) n -> p s kt n", p=P, s=2)
            )

        wr_sb = consts.tile([P, KT, EW], bf16)
        nc.gpsimd.dma_start(wr_sb[:], wr.rearrange("(kt p) n -> p kt n", p=P))

        mro_sb = consts.tile([P, RT * EW], f32)
        nc.gpsimd.dma_start(mro_sb[:], mro[:, :])
        mrw_sb = consts.tile([P, RT * EW], f32)
        nc.gpsimd.dma_start(mrw_sb[:], mrw[:, :])

        # token data resident in SBUF (staged once, like the weights)
        xg_sb = consts.tile([P, Tb * KT * P], bf16)
        nc.scalar.dma_start(xg_sb[:], xg_t[:, :])
        xr_sb = consts.tile([P, RT * KT * P], bf16)
        nc.sync.dma_start(xr_sb[:], xr[:, :])
        if T8 > 0:
            xg8_sb = consts.tile([P, T8 * KT * P], fp8)
            nc.gpsimd.dma_start(xg8_sb[:], xg8[:, :])

        rep_cm = None
        if repeat > 1:
            rep_cm = tc.For_i(0, repeat, 1)
            rep_cm.__enter__()

        # groups: (kind, g0, gt) with g0 a GLOBAL tile index; fp8 groups
        # follow the bf16 groups and never straddle the boundary.
        groups_b = [("b", g0, min(G, Tb - g0)) for g0 in range(0, Tb, G)]
        groups_8 = [("8", Tb + g0, min(G, T8 - g0)) for g0 in range(0, T8, G)]
        if GORDER == "mix":
            groups = []
            for i in range(max(len(groups_b), len(groups_8))):
                if i < len(groups_b):
                    groups.append(groups_b[i])
                if i < len(groups_8):
                    groups.append(groups_8[i])
        else:
            groups = groups_b + groups_8

        do_load = ABLATE not in ("noload", "pe", "penr")
        do_store = ABLATE not in ("nostore", "pe", "penr")
        do_pe = ABLATE != "dmaonly"
        do_router = do_pe and ABLATE not in ("norouter", "penr")
        kt_eff = KT // 2 if ABLATE == "halfk" else KT

        # router emission state machine: one tiny (LDW + 8-row matmul) pair at
        # a time, interleaved between 512-row expert matmuls so the router
        # stationary loads hide under the long streams
        r_state = {"pos": 0, "lp": None, "gsb": None}
        n_router_ops = RT * KT

        def router_chain(rtau, Lp, gsb):
            """Gate chain for one router tile (two masked reduce_max)."""
            Lm1 = gpool.tile([P, EW], f32, tag="lm1")
            nc.vector.tensor_add(Lm1[:], Lp[:], mro_sb[:, rtau * EW : (rtau + 1) * EW])
            Lm2 = gpool.tile([P, EW], f32, tag="lm2")
            nc.vector.tensor_add(Lm2[:], Lp[:], mrw_sb[:, rtau * EW : (rtau + 1) * EW])
            mo = gpool.tile([P, 1], f32, tag="mo")
            nc.vector.reduce_max(
                mo[:], Lm1[:].rearrange("p (g e) -> p g e", e=EW),
                axis=mybir.AxisListType.X,
            )
            so = gpool.tile([P, 1], f32, tag="so")
            nc.vector.reduce_max(
                so[:], Lm2[:].rearrange("p (g e) -> p g e", e=EW),
                axis=mybir.AxisListType.X,
            )
            nc.vector.tensor_sub(so[:], so[:], mo[:])
            nc.scalar.activation(
                gsb[:, rtau : rtau + 1], so[:], mybir.ActivationFunctionType.Sigmoid
            )

        def emit_router(n):
            """Emit up to n router matmul pairs (chains piggyback for free)."""
            if not do_router:
                return
            emitted = 0
            while r_state["pos"] < n_router_ops and emitted < n:
                pos = r_state["pos"]
                rtau, j = divmod(pos, KT)
                if j == 0:
                    r_state["lp"] = lpsum.tile([P, EW], f32, tag="lp", name="lp")
                nc.tensor.matmul(
                    r_state["lp"][:],
                    xr_sb[:, (rtau * KT + j) * P : (rtau * KT + j + 1) * P],
                    wr_sb[:, j, :],
                    start=(j == 0),
                    stop=(j == KT - 1),
                )
                if j == KT - 1:
                    router_chain(rtau, r_state["lp"], r_state["gsb"])
                r_state["pos"] = pos + 1
                emitted += 1

        def experts(kind, g0, gt, gsb):
            ysb = None
            if YSTORE != "psum":
                ysb = ypool.tile([P, gt * D], bf16, tag="ysb")
            if T8 > 0:
                x8v = xg8_sb[:].rearrange("p (t k c) -> p t k c", k=KT, c=P)
            for tau in range(gt if do_pe else 0):
                t = g0 + tau
                yph = [
                    ypsum.tile([P, NH], f32, tag="yph", name="yph")
                    for _ in range(2)
                ]
                if kind == "b":
                    slot = 0 if t < Sb else 1
                    for nh in range(2):
                        for j in range(kt_eff):
                            nc.tensor.matmul(
                                yph[nh][:],
                                xg_sb[:, (t * KT + j) * P : (t * KT + j + 1) * P],
                                w_sb[:, slot, j, ts(nh, NH)],
                                start=(j == 0),
                                stop=(j == kt_eff - 1),
                            )
                else:
                    t8 = t - Tb
                    slot = 0 if t8 < S8 else 1
                    kp_eff = kt_eff // 2
                    for nh in range(2):
                        for m in range(kp_eff):
                            nc.tensor.matmul(
                                yph[nh][:],
                                x8v[:, t8, 2 * m : 2 * m + 2, :],
                                w8_sb[:, slot, 2 * m : 2 * m + 2, ts(nh, NH)],
                                start=(m == 0),
                                stop=(m == kp_eff - 1),
                                perf_mode=mybir.MatmulPerfMode.DoubleRow,
                            )
                if YSTORE == "psum":
                    if do_store:
                        eng = nc.sync if t % 2 == 0 else nc.scalar
                        for nh in range(2):
                            eng.dma_start(
                                y[:, t * D + nh * NH : t * D + (nh + 1) * NH],
                                yph[nh][:],
                            )
                    else:
                        # keep psum consumed so the pool can recycle
                        junk = gpool.tile([P, 1], f32, tag="junk", name="junk")
                        nc.vector.reduce_max(junk[:], yph[0][:], axis=mybir.AxisListType.X)
                        nc.vector.reduce_max(junk[:], yph[1][:], axis=mybir.AxisListType.X)
                    continue
                for nh in range(2):
                    dsth = ysb[:, tau * D + nh * NH : tau * D + (nh + 1) * NH]
                    if nh == 0 or DRAINQ == "dve":
                        nc.vector.tensor_copy(dsth, yph[nh][:])
                    else:
                        nc.scalar.copy(dsth, yph[nh][:])
            def emit_store():
                eng = {"act": nc.scalar, "sync": nc.sync, "pool": nc.gpsimd}.get(YQ, nc.scalar)
                if YSTORE == "half":
                    for tau in range(gt):
                        for nh in range(2):
                            k = (g0 + tau) * 2 + nh + (1 if YXQ == "1" else 0)
                            e2 = nc.sync if k % 2 == 0 else nc.scalar
                            e2.dma_start(
                                y[:, (g0 + tau) * D + nh * NH : (g0 + tau) * D + (nh + 1) * NH],
                                ysb[:, tau * D + nh * NH : tau * D + (nh + 1) * NH],
                            )
                elif YSTORE == "tile":
                    for tau in range(gt):
                        qs = {"alt": [nc.sync, nc.scalar],
                              "alt3": [nc.sync, nc.scalar, nc.gpsimd],
                              "alt4": [nc.sync, nc.scalar, nc.gpsimd, nc.vector]}.get(YQ)
                        e2 = qs[(g0 + tau) % len(qs)] if qs else eng
                        e2.dma_start(
                            y[:, (g0 + tau) * D : (g0 + tau + 1) * D],
                            ysb[:, tau * D : (tau + 1) * D],
                        )
                else:
                    eng.dma_start(y[:, g0 * D : (g0 + gt) * D], ysb[:])

            if do_store and YSTORE != "psum":
                if YDELAY == "end":
                    pending_stores.append(emit_store)
                else:
                    emit_store()

        pending_stores = []
        gsb_cur = gspool.tile([P, RT], f32, tag="gsb", name="gsb")
        r_state["gsb"] = gsb_cur
        if not do_router:
            nc.gpsimd.memset(gsb_cur[:], 1.0)
        for i, (kind, g0, gt) in enumerate(groups):
            emit_router(RCHUNK)  # a few router tiles' worth per group
            experts(kind, g0, gt, gsb_cur)
            if i + 1 == len(groups):
                emit_router(n_router_ops)  # flush any remaining router work
                for st in pending_stores:
                    st()
                pending_stores.clear()
                if do_pe:
                    nc.sync.dma_start(gout[:, :], gsb_cur[:])

        if rep_cm is not None:
            rep_cm.__exit__(None, None, None)

    nc.compile()
    _BUILD_CACHE[key] = nc
    return nc


def _route(x, Wr, br):
    """Host routing in fp64: per-token top-2 expert ids + gates."""
    n_tokens = x.shape[0]
    logits = x.astype(np.float64) @ Wr.astype(np.float64) + br.astype(np.float64)
    i1 = np.argmax(logits, axis=1)
    l1 = logits[np.arange(n_tokens), i1]
    l2m = logits.copy()
    l2m[np.arange(n_tokens), i1] = -np.inf
    i2 = np.argmax(l2m, axis=1)
    l2 = logits[np.arange(n_tokens), i2]
    g1 = 1.0 / (1.0 + np.exp(l2 - l1))
    return i1, i2, g1, 1.0 - g1


def _solve_assignment(n_tiles_per_e):
    """Pick (T, S) and per-core (primary_e, secondary_e) so that 8 units of
    size S plus 8 units of size T-S cover the per-expert tile demands.
    Exact DP over (S-units used, (T-S)-units used).

    Returns (T, S, pri_experts[8], sec_experts[8]) or None."""
    total = int(sum(n_tiles_per_e))
    if total == 0:
        return 0, 0, [0] * N_CORES, [0] * N_CORES
    t_min = max(1, (total + N_CORES - 1) // N_CORES)
    for T in range(t_min, t_min + 4):
        for S in range((T + 1) // 2, T + 1):
            sz2 = T - S
            # per-expert candidate (p, q) unit counts, Pareto-minimal
            opts = []
            for ne in n_tiles_per_e:
                cand = []
                for p in range(N_CORES + 1):
                    need = ne - S * p
                    q = 0 if need <= 0 else (
                        (need + sz2 - 1) // sz2 if sz2 > 0 else None
                    )
                    if q is None or q > N_CORES:
                        continue
                    if any(p2 <= p and q2 <= q for p2, q2 in cand):
                        continue
                    cand = [(p2, q2) for p2, q2 in cand if not (p <= p2 and q <= q2)]
                    cand.append((p, q))
                if not cand:
                    cand = None
                opts.append(cand)
            if any(o is None for o in opts):
                continue
            # DP: state (pu, qu) -> choice list
            states = {(0, 0): []}
            for cand in opts:
                nxt = {}
                for (pu, qu), hist in states.items():
                    for p, q in cand:
                        k = (pu + p, qu + q)
                        if k[0] <= N_CORES and k[1] <= N_CORES and k not in nxt:
                            nxt[k] = hist + [(p, q)]
                states = nxt
            if not states:
                continue
            (pu, qu), hist = min(states.items(), key=lambda kv: kv[0])
            pri, sec = [], []
            for e, (p, q) in enumerate(hist):
                pri += [e] * p
                sec += [e] * q
            pri += [0] * (N_CORES - len(pri))  # leftover units: pure padding
            sec += [0] * (N_CORES - len(sec))
            return T, S, pri, sec
    return None


def _assign(idx_per_e):
    """Tile one precision class and distribute across cores.

    Returns (T, S, core_tiles) with core_tiles[c] a list of (expert, idx)."""
    tiles_per_e = [
        [idx[i : i + P] for i in range(0, len(idx), P)] for idx in idx_per_e
    ]
    n_tiles = [len(tl) for tl in tiles_per_e]
    sol = _solve_assignment(n_tiles)
    if sol is None:
        T = max(max(n_tiles), 1)
        S = T
        pri = list(range(N_CORES))
        sec = list(range(N_CORES))
    else:
        T, S, pri, sec = sol
    queues = [list(tl) for tl in tiles_per_e]
    core_tiles = [[] for _ in range(N_CORES)]
    for c in range(N_CORES):
        for e, cnt in ((pri[c], S), (sec[c], T - S)):
            for _ in range(cnt):
                idx = queues[e].pop(0) if queues[e] else np.empty(0, np.int64)
                core_tiles[c].append((e, idx))
    assert all(not q for q in queues), "assignment failed to place all tiles"
    return T, S, core_tiles


def _plan(x, Wr, br, W, b):
    i1, i2, g1, g2 = _route(x, Wr, br)
    n = x.shape[0]
    # token-expert pairs: (expert, token, gate)
    pairs_e = np.concatenate([i1, i2])
    pairs_t = np.concatenate([np.arange(n), np.arange(n)])
    pairs_g = np.concatenate([g1, g2])
    # per-expert flex split under a global error budget:
    # choose k_e (lowest-gate k_e pairs -> fp8) minimizing PE cost
    # 2*ceil(nb/128) + ceil(n8/128) subject to sum of fp8 g^2 <= budget
    budget = max(0.0, (ERR_TARGET**2 - ERR_BF16**2) / ERR_C)
    toks, gsq, f8s, his, ks = [], [], [], [], []
    for e in range(N_EXPERTS):
        m = pairs_e == e
        t_e = pairs_t[m]
        g_e = pairs_g[m]
        order = np.argsort(g_e, kind="stable")
        toks.append(t_e[order])
        gs = np.concatenate([[0.0], np.cumsum(g_e[order] ** 2)])
        gsq.append(gs)
        f8s.append(int((g_e < THETA_LO).sum()))
        his.append(int((g_e < THETA_HI).sum()))
        ks.append(f8s[-1])

    def cover_cost(kvec):
        """Per-core PE cost of the full covering for a split: 2*Tb + T8
        (bf16 tile = 16 matmuls, fp8 tile = 8). Returns (cost, Tb, T8)."""
        nb = [-(-(len(toks[e]) - kvec[e]) // P) for e in range(N_EXPERTS)]
        n8 = [-(-kvec[e] // P) for e in range(N_EXPERTS)]
        sb_ = _solve_assignment(nb)
        s8_ = _solve_assignment(n8)
        if sb_ is None or s8_ is None:
            return None
        return 2 * sb_[0] + s8_[0], sb_[0], s8_[0]

    # candidate splits: sharp thresholds + the per-expert greedy trajectory
    # (objective 2*ceil(nb/128)+ceil(n8/128)); evaluate each with the real
    # covering solver and keep the cheapest within the error budget.
    g_sorted = [np.sort(pairs_g[pairs_e == e]) for e in range(N_EXPERTS)]
    cands = []
    for th in np.arange(THETA_LO, THETA_HI + 1e-9, 0.00625):
        cands.append(
            ([int(np.searchsorted(g_sorted[e], th)) for e in range(N_EXPERTS)], True)
        )
    # uniform-m candidates: keep the 128*m highest-gate pairs per expert in
    # bf16 (exact Tb=m cover with zero fragmentation), everything else fp8
    for m_u in range(7, 13):
        kvec = [max(0, len(toks[e]) - 128 * m_u) for e in range(N_EXPERTS)]
        cands.append((kvec, False))

    def pcost(e, k):
        n_e = len(toks[e])
        return 2 * -(-(n_e - k) // P) + -(-k // P)

    kg = list(ks)
    cands.append((list(kg), True))
    while True:
        best = None
        for e in range(N_EXPERTS):
            c0 = pcost(e, kg[e])
            for k in range(kg[e] + 1, his[e] + 1):
                if pcost(e, k) < c0:
                    ds = gsq[e][k] - gsq[e][kg[e]]
                    dc = c0 - pcost(e, k)
                    if best is None or ds / dc < best[0]:
                        best = (ds / dc, e, k)
                    break
        if best is None:
            break
        kg[best[1]] = best[2]
        cands.append((list(kg), True))

    best = None
    for kvec, clamp in cands:
        if clamp:
            kvec = [min(max(kvec[e], f8s[e]), his[e]) for e in range(N_EXPERTS)]
        spent = sum(gsq[e][kvec[e]] for e in range(N_EXPERTS))
        if spent > budget:
            continue
        cc = cover_cost(kvec)
        if cc is None:
            continue
        key = (cc[0], spent)
        if best is None or key < best[0]:
            best = (key, kvec)
    assert best is not None, "no feasible precision split"
    ks = best[1]
    idxb = [np.sort(toks[e][ks[e] :]) for e in range(N_EXPERTS)]
    idx8 = [np.sort(toks[e][: ks[e]]) for e in range(N_EXPERTS)]
    Tb, Sb, tiles_b = _assign(idxb)
    T8, S8, tiles_8 = _assign(idx8)
    plan = {
        "build_args": {"Tb": Tb, "Sb": Sb, "T8": T8, "S8": S8},
        "tiles_b": tiles_b,
        "tiles_8": tiles_8,
        "n_tokens": n,
        "b": b,
        "i1": i1,
    }
    return plan


def _make_in_maps(x, Wr, br, W, b, plan):
    Tb, Sb = plan["build_args"]["Tb"], plan["build_args"]["Sb"]
    T8, S8 = plan["build_args"]["T8"], plan["build_args"]["S8"]
    T_all = Tb + T8
    x8_full = x.astype(ml_dtypes.float8_e4m3)
    i1 = plan["i1"]
    in_maps = []
    for c in range(N_CORES):
        xg = np.zeros((Tb * P, D), dtype=np.float32)
        for t, (e, idx) in enumerate(plan["tiles_b"][c]):
            if len(idx):
                xg[t * P : t * P + len(idx)] = x[idx]
        # partition-major: xg_t[p, t, j, c] = xg[t*128 + c, j*128 + p]
        xg_t = np.ascontiguousarray(
            xg.reshape(Tb, P, KT, P).transpose(3, 0, 2, 1).reshape(P, Tb * KT * P)
        ).astype(ml_dtypes.bfloat16)
        # router tokens: core-contiguous slab, plus per-token top-1 masks
        toks = np.arange(c * RT * P, (c + 1) * RT * P)
        xr_t = np.ascontiguousarray(
            x[toks].reshape(RT, P, KT, P).transpose(3, 0, 2, 1).reshape(P, RT * KT * P)
        ).astype(ml_dtypes.bfloat16)
        own = i1[toks].reshape(RT, P)
        tt, pp = np.mgrid[0:RT, 0:P]
        mro = np.broadcast_to(br, (RT, P, EW)).astype(np.float32).copy()
        mro[tt, pp, own] = -1e30
        mrw = np.full((RT, P, EW), -1e30, np.float32)
        mrw[tt, pp, own] = br[own]
        tb = plan["tiles_b"][c]
        e_pri = tb[0][0] if Tb else 0
        e_sec = tb[-1][0] if Tb else 0
        ws = np.concatenate([W[e_pri], W[e_sec]], axis=0).astype(ml_dtypes.bfloat16)
        im = {
            "xg_t": xg_t,
            "xr": xr_t,
            "ws": np.ascontiguousarray(ws),
            "wr": np.ascontiguousarray(Wr).astype(ml_dtypes.bfloat16),
            "mro": np.ascontiguousarray(mro.transpose(1, 0, 2).reshape(P, RT * EW)),
            "mrw": np.ascontiguousarray(mrw.transpose(1, 0, 2).reshape(P, RT * EW)),
        }
        if T8 > 0:
            t8 = plan["tiles_8"][c]
            xg8 = np.zeros((T8 * P, D), dtype=ml_dtypes.float8_e4m3)
            for t, (e, idx) in enumerate(t8):
                if len(idx):
                    xg8[t * P : t * P + len(idx)] = x8_full[idx]
            im["xg8"] = np.ascontiguousarray(
                xg8.reshape(T8, P, KT, P).transpose(3, 0, 2, 1).reshape(P, T8 * KT * P)
            )
            e8_pri = t8[0][0]
            e8_sec = t8[-1][0]
            ws8 = np.concatenate([W[e8_pri], W[e8_sec]], axis=0) * W8SCALE
            im["ws8"] = np.ascontiguousarray(ws8).astype(ml_dtypes.float8_e4m3)
        in_maps.append(im)
    return in_maps


def _prep(inputs):
    x = np.asarray(inputs["x"], dtype=np.float32)
    Wr = np.asarray(inputs["Wr"], dtype=np.float32)
    br = np.asarray(inputs["br"], dtype=np.float32)
    W = np.asarray(inputs["W"], dtype=np.float32)
    b = np.asarray(inputs["b"], dtype=np.float32)
    plan = _plan(x, Wr, br, W, b)
    # sanity: slot boundary must match data layout (tile t uses slot 0 iff t<S)
    for c in range(N_CORES):
        for tiles, S in ((plan["tiles_b"][c], plan["build_args"]["Sb"]),
                         (plan["tiles_8"][c], plan["build_args"]["S8"])):
            for t, (e, _) in enumerate(tiles):
                want = tiles[0][0] if t < S else tiles[-1][0]
                assert e == want, (c, t, e, want)
    in_maps = _make_in_maps(x, Wr, br, W, b, plan)
    return in_maps, plan


def kernel(**inputs) -> np.ndarray:
    global LAST_RESULTS
    in_maps, plan = _prep(inputs)
    nc = _build(**plan["build_args"])
    res = run_bass_kernel_spmd(nc, in_maps, core_ids=list(range(N_CORES)))
    LAST_RESULTS = res

    Tb = plan["build_args"]["Tb"]
    T8 = plan["build_args"]["T8"]
    T_all = Tb + T8
    n_tokens, b, i1 = plan["n_tokens"], plan["b"], plan["i1"]
    # primary gate per token from the per-core router outputs
    g_all = np.concatenate(
        [res.results[c]["gout"].T.reshape(RT * P).astype(np.float32)
         for c in range(N_CORES)]
    )
    out = np.zeros((n_tokens, D), dtype=np.float32)
    for c in range(N_CORES):
        # y [P, T_all*D]: y[p, t*D + f] = tile t, token-slot p, feature f
        ye = (
            res.results[c]["y"]
            .reshape(P, T_all, D)
            .transpose(1, 0, 2)
            .astype(np.float32)
        )
        for tiles, scale in ((plan["tiles_b"][c], 1.0),
                             (plan["tiles_8"][c], 1.0 / W8SCALE)):
            off = 0 if scale == 1.0 else Tb
            for t, (e, idx) in enumerate(tiles):
                nn = len(idx)
                if nn:
                    g = np.where(i1[idx] == e, g_all[idx], 1.0 - g_all[idx])
                    out[idx] += g[:, None] * (
                        ye[off + t, :nn] * scale + b[e][None, :]
                    )
    return out


# revision 50
# speedup vs baseline: 1.0320x; 1.0008x over previous
"""MoE (8 experts, top-2, d=1024, N=8192) on 8 trn2 NeuronCores.

Strategy (capacity-balanced expert-parallel, mixed precision):
 - Host computes routing (top-2 expert ids + gates per token, fp64 logits for
   stable ordering). Token-expert pairs are split by gate magnitude: pairs
   with gate < THETA are computed in fp8-e4m3 with DoubleRow matmuls (2x PE
   throughput, K=256 per instruction); the rest in bf16. Each precision
   class is chopped into 128-token tiles per expert and distributed so every
   core gets exactly Tb bf16 tiles and T8 fp8 tiles. Per precision, each
   core holds TWO expert weight slots (primary/secondary); tiles 0..S-1 use
   slot 0 and the rest slot 1 (S is compile-time, experts are per-core DATA).
   A covering solver picks (T, S) and the assignment per precision.
 - Device (per core, SPMD): router logits for all tiles from bf16 x
   (replicated router), gate g = sigmoid((l_own+br_own) - max_{e!=own}) via
   two masked reduce_max; expert matmul per tile with PSUM K-accumulation
   (bf16: 16 N=512 matmuls; fp8: 8 DoubleRow N=512 matmuls, K=256 each),
   unscaled y emitted bf16 (fp8 tiles carry a x32 weight scale).
 - Host combines (weighted all-to-all): out[idx] += g*(y*inv_scale + b[e]).
"""

import os
from contextlib import ExitStack

import ml_dtypes
import numpy as np

import concourse.bass as bass
import concourse.bacc as bacc
import concourse.mybir as mybir
import concourse.tile as tile
from concourse.bass import ts
from concourse.bass_utils import run_bass_kernel_spmd

N_EXPERTS = 8
TOP_K = 2
D = 1024
N_CORES = 8
P = 128  # partitions
KT = D // P  # number of K tiles (8)
NH = 512  # psum free-dim tile (one bank of fp32)
EW = N_EXPERTS  # logit row width
N_TOKENS = 8192
RT = N_TOKENS // (N_CORES * P)  # router tiles per core (token-contiguous)
G = int(os.environ.get("MOE_G", "3"))  # token tiles per group
W8SCALE = 32.0  # fp8 weight pre-scale (W entries ~ N(0, 1/32))

MM_DTYPE = "bf16"
# gate thresholds: pairs with gate < THETA_LO run fp8 DoubleRow, pairs with
# gate >= THETA_HI run bf16; pairs in between are "flex" and are assigned to
# whichever class minimizes per-expert tile cost (bf16 tile = 2x fp8 tile)
THETA_LO = float(os.environ.get("MOE_THETA_LO", "0.40"))
THETA_HI = float(os.environ.get("MOE_THETA_HI", "0.50"))
# error budget: keep total rel err <= ERR_TARGET using the calibrated model
# err^2 = ERR_BF16^2 + ERR_C * sum(g^2 over fp8 pairs)
ERR_TARGET = float(os.environ.get("MOE_ERR_TARGET", "0.0182"))
ERR_BF16 = 3.2e-3
ERR_C = 3.15e-7
# ablation for HW bottleneck decomposition (bench-only, breaks correctness):
#   noload / nostore / pe (neither) / norouter / penr / dmaonly (no compute)
ABLATE = os.environ.get("MOE_ABLATE", "")
BUFS = int(os.environ.get("MOE_BUFS", "3"))  # x/y pool double-buffering depth
RCHUNK = int(os.environ.get("MOE_RCHUNK", "24"))  # router pairs emitted per group
YSTORE = os.environ.get("MOE_YSTORE", "half")  # y store granularity: group|tile|half
YQ = os.environ.get("MOE_YQ", "alt")  # y store queue: act|sync|pool|alt (sync/act per tile)
YPBUFS = int(os.environ.get("MOE_YPBUFS", "6"))  # expert psum pool depth
LPBUFS = int(os.environ.get("MOE_LPBUFS", "2"))  # router psum pool depth
# "end": queue all y stores at body end so their SBUF reads overlap the next
# iteration's router window instead of the port-saturated expert streams
YDELAY = os.environ.get("MOE_YDELAY", "0")
DRAINQ = os.environ.get("MOE_DRAINQ", "mix")  # psum drain engines: mix|dve
YXQ = os.environ.get("MOE_YXQ", "0")  # "1": cross-pair store queue vs drain engine
GORDER = os.environ.get("MOE_GORDER", "seq")  # group order: seq (b..b,8..8) | mix (round-robin)

LAST_RESULTS = None  # stash of BassKernelResults for test harness inspection

_BUILD_CACHE = {}


def _build(Tb: int, Sb: int, T8: int, S8: int, repeat: int = 1):
    """Build the SPMD Bass module: Tb bf16 + T8 fp8 token tiles per core."""
    key = (Tb, Sb, T8, S8, repeat, G, ABLATE, BUFS, RCHUNK, YSTORE, YQ, YPBUFS, LPBUFS, YDELAY, DRAINQ, YXQ, GORDER)
    if key in _BUILD_CACHE:
        return _BUILD_CACHE[key]

    f32 = mybir.dt.float32
    bf16 = mybir.dt.bfloat16
    fp8 = mybir.dt.float8e4
    T_all = Tb + T8

    nc = bacc.Bacc(None, target_bir_lowering=False)
    # bf16 tokens for the bf16 expert tiles:
    # xg_t[p, ((t*KT + j)*P + c)] = x[tile t slot c, j*128 + p]
    xg_t = nc.declare_dram_parameter("xg_t", [P, Tb * KT * P], bf16, isOutput=False)
    ws = nc.declare_dram_parameter("ws", [2 * D, D], bf16, isOutput=False)
    wr = nc.declare_dram_parameter("wr", [D, EW], bf16, isOutput=False)
    if T8 > 0:
        # fp8 tokens for fp8 tiles only (expert stationary)
        xg8 = nc.declare_dram_parameter("xg8", [P, T8 * KT * P], fp8, isOutput=False)
        ws8 = nc.declare_dram_parameter("ws8", [2 * D, D], fp8, isOutput=False)
    # router tokens: core-contiguous (token c*1024 + t*128 + p in slot
    # [p, (t j c)]); per-token masks: mro masks the token's top-1 column,
    # mrw masks all but the top-1 column (rows differ per partition)
    xr = nc.declare_dram_parameter("xr", [P, RT * KT * P], bf16, isOutput=False)
    mro = nc.declare_dram_parameter("mro", [P, RT * EW], f32, isOutput=False)
    mrw = nc.declare_dram_parameter("mrw", [P, RT * EW], f32, isOutput=False)
    # outputs (y tiled [T_all, 128 tok, 1024 feat]; primary gates per token)
    y_dt = f32 if YSTORE == "psum" else bf16
    y = nc.declare_dram_parameter("y", [P, T_all * D], y_dt, isOutput=True)
    gout = nc.declare_dram_parameter("gout", [P, RT], f32, isOutput=True)

    with tile.TileContext(nc) as tc, ExitStack() as ctx:
        consts = ctx.enter_context(tc.tile_pool(name="consts", bufs=1))
        gpool = ctx.enter_context(tc.tile_pool(name="gates", bufs=2))
        gspool = ctx.enter_context(tc.tile_pool(name="gsb", bufs=2))
        ypool = ctx.enter_context(tc.tile_pool(name="y", bufs=BUFS))
        lpsum = ctx.enter_context(
            tc.tile_pool(name="lpsum", bufs=LPBUFS, space=bass.MemorySpace.PSUM)
        )
        ypsum = ctx.enter_context(
            tc.tile_pool(name="ypsum", bufs=YPBUFS, space=bass.MemorySpace.PSUM)
        )

        # ---- constants / weights resident in SBUF ----
        # staging DMAs spread across three queues so startup overlaps
        w_sb = consts.tile([P, 2, KT, D], bf16)
        nc.sync.dma_start(w_sb[:], ws.rearrange("(s kt p) n -> p s kt n", p=P, s=2))
        if T8 > 0:
            w8_sb = consts.tile([P, 2, KT, D], fp8)
            nc.scalar.dma_start(
                w8_sb[:], ws8.rearrange("(s kt p) n -> p s kt n", p=P, s=2)
            )

        wr_sb = consts.tile([P, KT, EW], bf16)
        nc.gpsimd.dma_start(wr_sb[:], wr.rearrange("(kt p) n -> p kt n", p=P))

        mro_sb = consts.tile([P, RT * EW], f32)
        nc.gpsimd.dma_start(mro_sb[:], mro[:, :])
        mrw_sb = consts.tile([P, RT * EW], f32)
        nc.gpsimd.dma_start(mrw_sb[:], mrw[:, :])

        # token data resident in SBUF (staged once, like the weights)
        xg_sb = consts.tile([P, Tb * KT * P], bf16)
        nc.scalar.dma_start(xg_sb[:], xg_t[:, :])
        xr_sb = consts.tile([P, RT * KT * P], bf16)
        nc.sync.dma_start(xr_sb[:], xr[:, :])
        if T8 > 0:
            xg8_sb = consts.tile([P, T8 * KT * P], fp8)
            nc.gpsimd.dma_start(xg8_sb[:], xg8[:, :])

        rep_cm = None
        if repeat > 1:
            rep_cm = tc.For_i(0, repeat, 1)
            rep_cm.__enter__()

        # groups: (kind, g0, gt) with g0 a GLOBAL tile index; fp8 groups
        # follow the bf16 groups and never straddle the boundary.
        groups_b = [("b", g0, min(G, Tb - g0)) for g0 in range(0, Tb, G)]
        groups_8 = [("8", Tb + g0, min(G, T8 - g0)) for g0 in range(0, T8, G)]
        if GORDER == "mix":
            groups = []
            for i in range(max(len(groups_b), len(groups_8))):
                if i < len(groups_b):
                    groups.append(groups_b[i])
                if i < len(groups_8):
                    groups.append(groups_8[i])
        else:
            groups = groups_b + groups_8

        do_load = ABLATE not in ("noload", "pe", "penr")
        do_store = ABLATE not in ("nostore", "pe", "penr")
        do_pe = ABLATE != "dmaonly"
        do_router = do_pe and ABLATE not in ("norouter", "penr")
        kt_eff = KT // 2 if ABLATE == "halfk" else KT

        # router emission state machine: one tiny (LDW + 8-row matmul) pair at
        # a time, interleaved between 512-row expert matmuls so the router
        # stationary loads hide under the long streams
        r_state = {"pos": 0, "lp": None, "gsb": None}
        n_router_ops = RT * KT

        def router_chain(rtau, Lp, gsb):
            """Gate chain for one router tile (two masked reduce_max)."""
            Lm1 = gpool.tile([P, EW], f32, tag="lm1")
            nc.vector.tensor_add(Lm1[:], Lp[:], mro_sb[:, rtau * EW : (rtau + 1) * EW])
            Lm2 = gpool.tile([P, EW], f32, tag="lm2")
            nc.vector.tensor_add(Lm2[:], Lp[:], mrw_sb[:, rtau * EW : (rtau + 1) * EW])
            mo = gpool.tile([P, 1], f32, tag="mo")
            nc.vector.reduce_max(
                mo[:], Lm1[:].rearrange("p (g e) -> p g e", e=EW),
                axis=mybir.AxisListType.X,
            )
            so = gpool.tile([P, 1], f32, tag="so")
            nc.vector.reduce_max(
                so[:], Lm2[:].rearrange("p (g e) -> p g e", e=EW),
                axis=mybir.AxisListType.X,
            )
            nc.vector.tensor_sub(so[:], so[:], mo[:])
            nc.scalar.activation(
                gsb[:, rtau : rtau + 1], so[:], mybir.ActivationFunctionType.Sigmoid
            )

        def emit_router(n):
            """Emit up to n router matmul pairs (chains piggyback for free)."""
            if not do_router:
                return
            emitted = 0
            while r_state["pos"] < n_router_ops and emitted < n:
                pos = r_state["pos"]
                rtau, j = divmod(pos, KT)
                if j == 0:
                    r_state["lp"] = lpsum.tile([P, EW], f32, tag="lp", name="lp")
                nc.tensor.matmul(
                    r_state["lp"][:],
                    xr_sb[:, (rtau * KT + j) * P : (rtau * KT + j + 1) * P],
                    wr_sb[:, j, :],
                    start=(j == 0),
                    stop=(j == KT - 1),
                )
                if j == KT - 1:
                    router_chain(rtau, r_state["lp"], r_state["gsb"])
                r_state["pos"] = pos + 1
                emitted += 1

        def experts(kind, g0, gt, gsb):
            ysb = None
            if YSTORE != "psum":
                ysb = ypool.tile([P, gt * D], bf16, tag="ysb")
            if T8 > 0:
                x8v = xg8_sb[:].rearrange("p (t k c) -> p t k c", k=KT, c=P)
            for tau in range(gt if do_pe else 0):
                t = g0 + tau
                yph = [
                    ypsum.tile([P, NH], f32, tag="yph", name="yph")
                    for _ in range(2)
                ]
                if kind == "b":
                    slot = 0 if t < Sb else 1
                    for nh in range(2):
                        for j in range(kt_eff):
                            nc.tensor.matmul(
                                yph[nh][:],
                                xg_sb[:, (t * KT + j) * P : (t * KT + j + 1) * P],
                                w_sb[:, slot, j, ts(nh, NH)],
                                start=(j == 0),
                                stop=(j == kt_eff - 1),
                            )
                else:
                    t8 = t - Tb
                    slot = 0 if t8 < S8 else 1
                    kp_eff = kt_eff // 2
                    for nh in range(2):
                        for m in range(kp_eff):
                            nc.tensor.matmul(
                                yph[nh][:],
                                x8v[:, t8, 2 * m : 2 * m + 2, :],
                                w8_sb[:, slot, 2 * m : 2 * m + 2, ts(nh, NH)],
                                start=(m == 0),
                                stop=(m == kp_eff - 1),
                                perf_mode=mybir.MatmulPerfMode.DoubleRow,
                            )
                if YSTORE == "psum":
                    if do_store:
                        eng = nc.sync if t % 2 == 0 else nc.scalar
                        for nh in range(2):
                            eng.dma_start(
                                y[:, t * D + nh * NH : t * D + (nh + 1) * NH],
                                yph[nh][:],
                            )
                    else:
                        # keep psum consumed so the pool can recycle
                        junk = gpool.tile([P, 1], f32, tag="junk", name="junk")
                        nc.vector.reduce_max(junk[:], yph[0][:], axis=mybir.AxisListType.X)
                        nc.vector.reduce_max(junk[:], yph[1][:], axis=mybir.AxisListType.X)
                    continue
                for nh in range(2):
                    dsth = ysb[:, tau * D + nh * NH : tau * D + (nh + 1) * NH]
                    if nh == 0 or DRAINQ == "dve":
                        nc.vector.tensor_copy(dsth, yph[nh][:])
                    else:
                        nc.scalar.copy(dsth, yph[nh][:])
            def emit_store():
                eng = {"act": nc.scalar, "sync": nc.sync, "pool": nc.gpsimd}.get(YQ, nc.scalar)
                if YSTORE == "quarter":
                    for tau in range(gt):
                        for q in range(4):
                            k = (g0 + tau) * 4 + q
                            e2 = nc.sync if k % 2 == 0 else nc.scalar
                            e2.dma_start(
                                y[:, (g0 + tau) * D + q * 256 : (g0 + tau) * D + (q + 1) * 256],
                                ysb[:, tau * D + q * 256 : tau * D + (q + 1) * 256],
                            )
                elif YSTORE == "half":
                    for tau in range(gt):
                        for nh in range(2):
                            k = (g0 + tau) * 2 + nh + (1 if YXQ == "1" else 0)
                            e2 = nc.sync if k % 2 == 0 else nc.scalar
                            e2.dma_start(
                                y[:, (g0 + tau) * D + nh * NH : (g0 + tau) * D + (nh + 1) * NH],
                                ysb[:, tau * D + nh * NH : tau * D + (nh + 1) * NH],
                            )
                elif YSTORE == "tile":
                    for tau in range(gt):
                        qs = {"alt": [nc.sync, nc.scalar],
                              "alt3": [nc.sync, nc.scalar, nc.gpsimd],
                              "alt4": [nc.sync, nc.scalar, nc.gpsimd, nc.vector]}.get(YQ)
                        e2 = qs[(g0 + tau) % len(qs)] if qs else eng
                        e2.dma_start(
                            y[:, (g0 + tau) * D : (g0 + tau + 1) * D],
                            ysb[:, tau * D : (tau + 1) * D],
                        )
                else:
                    eng.dma_start(y[:, g0 * D : (g0 + gt) * D], ysb[:])

            if do_store and YSTORE != "psum":
                if YDELAY == "end":
                    pending_stores.append(emit_store)
                else:
                    emit_store()

        pending_stores = []
        gsb_cur = gspool.tile([P, RT], f32, tag="gsb", name="gsb")
        r_state["gsb"] = gsb_cur
        if not do_router:
            nc.gpsimd.memset(gsb_cur[:], 1.0)
        for i, (kind, g0, gt) in enumerate(groups):
            emit_router(RCHUNK)  # a few router tiles' worth per group
            experts(kind, g0, gt, gsb_cur)
            if i + 1 == len(groups):
                emit_router(n_router_ops)  # flush any remaining router work
                for st in pending_stores:
                    st()
                pending_stores.clear()
                if do_pe:
                    nc.sync.dma_start(gout[:, :], gsb_cur[:])

        if rep_cm is not None:
            rep_cm.__exit__(None, None, None)

    nc.compile()
    _BUILD_CACHE[key] = nc
    return nc


def _route(x, Wr, br):
    """Host routing in fp64: per-token top-2 expert ids + gates."""
    n_tokens = x.shape[0]
    logits = x.astype(np.float64) @ Wr.astype(np.float64) + br.astype(np.float64)
    i1 = np.argmax(logits, axis=1)
    l1 = logits[np.arange(n_tokens), i1]
    l2m = logits.copy()
    l2m[np.arange(n_tokens), i1] = -np.inf
    i2 = np.argmax(l2m, axis=1)
    l2 = logits[np.arange(n_tokens), i2]
    g1 = 1.0 / (1.0 + np.exp(l2 - l1))
    return i1, i2, g1, 1.0 - g1


def _solve_assignment(n_tiles_per_e):
    """Pick (T, S) and per-core (primary_e, secondary_e) so that 8 units of
    size S plus 8 units of size T-S cover the per-expert tile demands.
    Exact DP over (S-units used, (T-S)-units used).

    Returns (T, S, pri_experts[8], sec_experts[8]) or None."""
    total = int(sum(n_tiles_per_e))
    if total == 0:
        return 0, 0, [0] * N_CORES, [0] * N_CORES
    t_min = max(1, (total + N_CORES - 1) // N_CORES)
    for T in range(t_min, t_min + 4):
        for S in range((T + 1) // 2, T + 1):
            sz2 = T - S
            # per-expert candidate (p, q) unit counts, Pareto-minimal
            opts = []
            for ne in n_tiles_per_e:
                cand = []
                for p in range(N_CORES + 1):
                    need = ne - S * p
                    q = 0 if need <= 0 else (
                        (need + sz2 - 1) // sz2 if sz2 > 0 else None
                    )
                    if q is None or q > N_CORES:
                        continue
                    if any(p2 <= p and q2 <= q for p2, q2 in cand):
                        continue
                    cand = [(p2, q2) for p2, q2 in cand if not (p <= p2 and q <= q2)]
                    cand.append((p, q))
                if not cand:
                    cand = None
                opts.append(cand)
            if any(o is None for o in opts):
                continue
            # DP: state (pu, qu) -> choice list
            states = {(0, 0): []}
            for cand in opts:
                nxt = {}
                for (pu, qu), hist in states.items():
                    for p, q in cand:
                        k = (pu + p, qu + q)
                        if k[0] <= N_CORES and k[1] <= N_CORES and k not in nxt:
                            nxt[k] = hist + [(p, q)]
                states = nxt
            if not states:
                continue
            (pu, qu), hist = min(states.items(), key=lambda kv: kv[0])
            pri, sec = [], []
            for e, (p, q) in enumerate(hist):
                pri += [e] * p
                sec += [e] * q
            pri += [0] * (N_CORES - len(pri))  # leftover units: pure padding
            sec += [0] * (N_CORES - len(sec))
            return T, S, pri, sec
    return None


def _assign(idx_per_e):
    """Tile one precision class and distribute across cores.

    Returns (T, S, core_tiles) with core_tiles[c] a list of (expert, idx)."""
    tiles_per_e = [
        [idx[i : i + P] for i in range(0, len(idx), P)] for idx in idx_per_e
    ]
    n_tiles = [len(tl) for tl in tiles_per_e]
    sol = _solve_assignment(n_tiles)
    if sol is None:
        T = max(max(n_tiles), 1)
        S = T
        pri = list(range(N_CORES))
        sec = list(range(N_CORES))
    else:
        T, S, pri, sec = sol
    queues = [list(tl) for tl in tiles_per_e]
    core_tiles = [[] for _ in range(N_CORES)]
    for c in range(N_CORES):
        for e, cnt in ((pri[c], S), (sec[c], T - S)):
            for _ in range(cnt):
                idx = queues[e].pop(0) if queues[e] else np.empty(0, np.int64)
                core_tiles[c].append((e, idx))
    assert all(not q for q in queues), "assignment failed to place all tiles"
    return T, S, core_tiles


def _plan(x, Wr, br, W, b):
    i1, i2, g1, g2 = _route(x, Wr, br)
    n = x.shape[0]
    # token-expert pairs: (expert, token, gate)
    pairs_e = np.concatenate([i1, i2])
    pairs_t = np.concatenate([np.arange(n), np.arange(n)])
    pairs_g = np.concatenate([g1, g2])
    # per-expert flex split under a global error budget:
    # choose k_e (lowest-gate k_e pairs -> fp8) minimizing PE cost
    # 2*ceil(nb/128) + ceil(n8/128) subject to sum of fp8 g^2 <= budget
    budget = max(0.0, (ERR_TARGET**2 - ERR_BF16**2) / ERR_C)
    toks, gsq, f8s, his, ks = [], [], [], [], []
    for e in range(N_EXPERTS):
        m = pairs_e == e
        t_e = pairs_t[m]
        g_e = pairs_g[m]
        order = np.argsort(g_e, kind="stable")
        toks.append(t_e[order])
        gs = np.concatenate([[0.0], np.cumsum(g_e[order] ** 2)])
        gsq.append(gs)
        f8s.append(int((g_e < THETA_LO).sum()))
        his.append(int((g_e < THETA_HI).sum()))
        ks.append(f8s[-1])

    def cover_cost(kvec):
        """Per-core PE cost of the full covering for a split: 2*Tb + T8
        (bf16 tile = 16 matmuls, fp8 tile = 8). Returns (cost, Tb, T8)."""
        nb = [-(-(len(toks[e]) - kvec[e]) // P) for e in range(N_EXPERTS)]
        n8 = [-(-kvec[e] // P) for e in range(N_EXPERTS)]
        sb_ = _solve_assignment(nb)
        s8_ = _solve_assignment(n8)
        if sb_ is None or s8_ is None:
            return None
        return 2 * sb_[0] + s8_[0], sb_[0], s8_[0]

    # candidate splits: sharp thresholds + the per-expert greedy trajectory
    # (objective 2*ceil(nb/128)+ceil(n8/128)); evaluate each with the real
    # covering solver and keep the cheapest within the error budget.
    g_sorted = [np.sort(pairs_g[pairs_e == e]) for e in range(N_EXPERTS)]
    cands = []
    for th in np.arange(THETA_LO, THETA_HI + 1e-9, 0.00625):
        cands.append(
            ([int(np.searchsorted(g_sorted[e], th)) for e in range(N_EXPERTS)], True)
        )
    # uniform-m candidates: keep the 128*m highest-gate pairs per expert in
    # bf16 (exact Tb=m cover with zero fragmentation), everything else fp8
    for m_u in range(7, 13):
        kvec = [max(0, len(toks[e]) - 128 * m_u) for e in range(N_EXPERTS)]
        cands.append((kvec, False))

    def pcost(e, k):
        n_e = len(toks[e])
        return 2 * -(-(n_e - k) // P) + -(-k // P)

    kg = list(ks)
    cands.append((list(kg), True))
    while True:
        best = None
        for e in range(N_EXPERTS):
            c0 = pcost(e, kg[e])
            for k in range(kg[e] + 1, his[e] + 1):
                if pcost(e, k) < c0:
                    ds = gsq[e][k] - gsq[e][kg[e]]
                    dc = c0 - pcost(e, k)
                    if best is None or ds / dc < best[0]:
                        best = (ds / dc, e, k)
                    break
        if best is None:
            break
        kg[best[1]] = best[2]
        cands.append((list(kg), True))

    best = None
    for kvec, clamp in cands:
        if clamp:
            kvec = [min(max(kvec[e], f8s[e]), his[e]) for e in range(N_EXPERTS)]
        spent = sum(gsq[e][kvec[e]] for e in range(N_EXPERTS))
        if spent > budget:
            continue
        cc = cover_cost(kvec)
        if cc is None:
            continue
        key = (cc[0], spent)
        if best is None or key < best[0]:
            best = (key, kvec)
    assert best is not None, "no feasible precision split"
    ks = best[1]
    idxb = [np.sort(toks[e][ks[e] :]) for e in range(N_EXPERTS)]
    idx8 = [np.sort(toks[e][: ks[e]]) for e in range(N_EXPERTS)]
    Tb, Sb, tiles_b = _assign(idxb)
    T8, S8, tiles_8 = _assign(idx8)
    plan = {
        "build_args": {"Tb": Tb, "Sb": Sb, "T8": T8, "S8": S8},
        "tiles_b": tiles_b,
        "tiles_8": tiles_8,
        "n_tokens": n,
        "b": b,
        "i1": i1,
    }
    return plan


def _make_in_maps(x, Wr, br, W, b, plan):
    Tb, Sb = plan["build_args"]["Tb"], plan["build_args"]["Sb"]
    T8, S8 = plan["build_args"]["T8"], plan["build_args"]["S8"]
    T_all = Tb + T8
    x8_full = x.astype(ml_dtypes.float8_e4m3)
    i1 = plan["i1"]
    in_maps = []
    for c in range(N_CORES):
        xg = np.zeros((Tb * P, D), dtype=np.float32)
        for t, (e, idx) in enumerate(plan["tiles_b"][c]):
            if len(idx):
                xg[t * P : t * P + len(idx)] = x[idx]
        # partition-major: xg_t[p, t, j, c] = xg[t*128 + c, j*128 + p]
        xg_t = np.ascontiguousarray(
            xg.reshape(Tb, P, KT, P).transpose(3, 0, 2, 1).reshape(P, Tb * KT * P)
        ).astype(ml_dtypes.bfloat16)
        # router tokens: core-contiguous slab, plus per-token top-1 masks
        toks = np.arange(c * RT * P, (c + 1) * RT * P)
        xr_t = np.ascontiguousarray(
            x[toks].reshape(RT, P, KT, P).transpose(3, 0, 2, 1).reshape(P, RT * KT * P)
        ).astype(ml_dtypes.bfloat16)
        own = i1[toks].reshape(RT, P)
        tt, pp = np.mgrid[0:RT, 0:P]
        mro = np.broadcast_to(br, (RT, P, EW)).astype(np.float32).copy()
        mro[tt, pp, own] = -1e30
        mrw = np.full((RT, P, EW), -1e30, np.float32)
        mrw[tt, pp, own] = br[own]
        tb = plan["tiles_b"][c]
        e_pri = tb[0][0] if Tb else 0
        e_sec = tb[-1][0] if Tb else 0
        ws = np.concatenate([W[e_pri], W[e_sec]], axis=0).astype(ml_dtypes.bfloat16)
        im = {
            "xg_t": xg_t,
            "xr": xr_t,
            "ws": np.ascontiguousarray(ws),
            "wr": np.ascontiguousarray(Wr).astype(ml_dtypes.bfloat16),
            "mro": np.ascontiguousarray(mro.transpose(1, 0, 2).reshape(P, RT * EW)),
            "mrw": np.ascontiguousarray(mrw.transpose(1, 0, 2).reshape(P, RT * EW)),
        }
        if T8 > 0:
            t8 = plan["tiles_8"][c]
            xg8 = np.zeros((T8 * P, D), dtype=ml_dtypes.float8_e4m3)
            for t, (e, idx) in enumerate(t8):
                if len(idx):
                    xg8[t * P : t * P + len(idx)] = x8_full[idx]
            im["xg8"] = np.ascontiguousarray(
                xg8.reshape(T8, P, KT, P).transpose(3, 0, 2, 1).reshape(P, T8 * KT * P)
            )
            e8_pri = t8[0][0]
            e8_sec = t8[-1][0]
            ws8 = np.concatenate([W[e8_pri], W[e8_sec]], axis=0) * W8SCALE
            im["ws8"] = np.ascontiguousarray(ws8).astype(ml_dtypes.float8_e4m3)
        in_maps.append(im)
    return in_maps


def _prep(inputs):
    x = np.asarray(inputs["x"], dtype=np.float32)
    Wr = np.asarray(inputs["Wr"], dtype=np.float32)
    br = np.asarray(inputs["br"], dtype=np.float32)
    W = np.asarray(inputs["W"], dtype=np.float32)
    b = np.asarray(inputs["b"], dtype=np.float32)
    plan = _plan(x, Wr, br, W, b)
    # sanity: slot boundary must match data layout (tile t uses slot 0 iff t<S)
    for c in range(N_CORES):
        for tiles, S in ((plan["tiles_b"][c], plan["build_args"]["Sb"]),
                         (plan["tiles_8"][c], plan["build_args"]["S8"])):
            for t, (e, _) in enumerate(tiles):
                want = tiles[0][0] if t < S else tiles[-1][0]
                assert e == want, (c, t, e, want)
    in_maps = _make_in_maps(x, Wr, br, W, b, plan)
    return in_maps, plan


def kernel(**inputs) -> np.ndarray:
    global LAST_RESULTS
    in_maps, plan = _prep(inputs)
    nc = _build(**plan["build_args"])
    res = run_bass_kernel_spmd(nc, in_maps, core_ids=list(range(N_CORES)))
    LAST_RESULTS = res

    Tb = plan["build_args"]["Tb"]
    T8 = plan["build_args"]["T8"]
    T_all = Tb + T8
    n_tokens, b, i1 = plan["n_tokens"], plan["b"], plan["i1"]
    # primary gate per token from the per-core router outputs
    g_all = np.concatenate(
        [res.results[c]["gout"].T.reshape(RT * P).astype(np.float32)
         for c in range(N_CORES)]
    )
    out = np.zeros((n_tokens, D), dtype=np.float32)
    for c in range(N_CORES):
        # y [P, T_all*D]: y[p, t*D + f] = tile t, token-slot p, feature f
        ye = (
            res.results[c]["y"]
            .reshape(P, T_all, D)
            .transpose(1, 0, 2)
            .astype(np.float32)
        )
        for tiles, scale in ((plan["tiles_b"][c], 1.0),
                             (plan["tiles_8"][c], 1.0 / W8SCALE)):
            off = 0 if scale == 1.0 else Tb
            for t, (e, idx) in enumerate(tiles):
                nn = len(idx)
                if nn:
                    g = np.where(i1[idx] == e, g_all[idx], 1.0 - g_all[idx])
                    out[idx] += g[:, None] * (
                        ye[off + t, :nn] * scale + b[e][None, :]
                    )
    return out
